# revision 18
# baseline (speedup 1.0000x reference)
"""Multi-head attention (B=4, S=2048, D=1024, H=16, causal) on 8 TRN2 NeuronCores.

Sharding: batch (4) x head-group (2 groups of 8 heads) = 8 cores.
Megatron-style: wq/wk/wv column-parallel, wo row-parallel; the 2-way partial-sum
of the row-parallel output projection is folded into the host-side unshard.

Per-core algorithm (heads h in the core's group, q-chunks of 512 queries):
  QT[dk, s], KT[dk, s] = (x @ w + b)^T via PE matmuls on host-pre-transposed
  inputs; V[s, dv] likewise, with 64 ones-columns appended per head so that
  the PV matmul also produces softmax denominators.
  scoresT[k, q] = KT-slices x QT (two heads packed in the 128-partition dim,
  concurrent via PE row tiling since dk=64).
  E = exp(scoresT/8) on ACT (no max-subtraction needed: scores ~ N(0,1)).
  Causality: fully-masked key-blocks are never computed; diagonal-crossing
  blocks are trapezoid-sliced to their live q-range and only the leading
  128-column triangle gets a mask multiply.
  ctxT[dv, q] accumulates V-slices x E in PSUM; rows 64:128 = sum(E).
  The 64 ones-columns make the PV matmul emit the softmax denominator
  replicated across output partitions 64:128, so normalization is a pure
  DVE chain: copy -> approx-reciprocal -> multiply.
  y_partial[s, do] = sum over head-pairs of ctxT-slices x wo-rows (PSUM accum).

All matmul operands are bf16 (accumulation stays fp32 in PSUM); softmax
denominators, reciprocals and the final output stay fp32.

Scheduling: engines execute their instruction streams in order, so emission
order is the schedule. Attention is software-pipelined (scores of block n+1
issue before the PV of block n, hiding the ACT exp latency), and the next
chunk's projection + previous chunk's output-projection matmuls are dripped
between attention blocks as PE filler so the tensor engine never idles long
enough for the HAM clock gate to re-throttle it to 1.2 GHz.
"""
import sys
import numpy as np
import ml_dtypes

sys.path.insert(0, "/opt/trn_rl_repo")

from contextlib import ExitStack

import concourse.bacc as bacc
import concourse.tile as tile
from concourse import mybir
from concourse.bass_utils import run_bass_kernel_spmd

F32 = mybir.dt.float32
BF16 = mybir.dt.bfloat16
NP_BF16 = ml_dtypes.bfloat16

B, S, D, H = 4, 2048, 1024, 16
DK = D // H          # 64
HG = H // 2          # 8 heads per core
DG = HG * DK         # 512 columns per core group
SC = 512             # query-chunk width
KB = 128             # key-block height
N_SC = S // SC       # 4
N_KB = S // KB       # 16
N_DM = D // 128      # 8 contraction tiles for projections
N_PAIR = HG // 2     # 4 head pairs per core
EXPSCALE = 1.0 / 8.0  # 1/sqrt(DK)


def build_program():
    """Emit the SPMD Bass program (identical on all 8 cores)."""
    nc = bacc.Bacc("TRN2", target_bir_lowering=False, debug=False)

    qT_in = nc.dram_tensor("qT", [D, S], BF16, kind="ExternalInput").ap()
    kT_in = nc.dram_tensor("kT", [D, S], BF16, kind="ExternalInput").ap()
    vT_in = nc.dram_tensor("vT", [D, S], BF16, kind="ExternalInput").ap()
    wq_in = nc.dram_tensor("wq", [D, DG], BF16, kind="ExternalInput").ap()
    wk_in = nc.dram_tensor("wk", [D, DG], BF16, kind="ExternalInput").ap()
    wv_in = nc.dram_tensor("wv", [D, DG], BF16, kind="ExternalInput").ap()
    wo_in = nc.dram_tensor("wo", [DG, D], BF16, kind="ExternalInput").ap()
    bq_in = nc.dram_tensor("bqT", [DG, 1], F32, kind="ExternalInput").ap()
    bk_in = nc.dram_tensor("bkT", [DG, 1], F32, kind="ExternalInput").ap()
    bv_in = nc.dram_tensor("bv", [1, DG], F32, kind="ExternalInput").ap()
    # leading-triangle causal mask: mask[ki, h, qj] = (qj >= ki), [128, 2, 128]
    mask_in = nc.dram_tensor("masks", [KB, 2, KB], BF16, kind="ExternalInput").ap()
    y_out = nc.dram_tensor("y", [S, D], BF16, kind="ExternalOutput").ap()

    with tile.TileContext(nc) as tc, ExitStack() as ctx:
        stage = ctx.enter_context(tc.tile_pool(name="stage", bufs=48))
        wpool = ctx.enter_context(tc.tile_pool(name="wpool", bufs=24))
        wopool = ctx.enter_context(tc.tile_pool(name="wopool", bufs=1))
        qtpool = ctx.enter_context(tc.tile_pool(name="qtpool", bufs=8))
        ktpool = ctx.enter_context(tc.tile_pool(name="ktpool", bufs=1))
        vpool = ctx.enter_context(tc.tile_pool(name="vpool", bufs=1))
        epool = ctx.enter_context(tc.tile_pool(name="epool", bufs=4))
        cpool = ctx.enter_context(tc.tile_pool(name="cpool", bufs=12))
        mpool = ctx.enter_context(tc.tile_pool(name="mpool", bufs=1))
        ypool = ctx.enter_context(tc.tile_pool(name="ypool", bufs=2))
        rpool = ctx.enter_context(tc.tile_pool(name="rpool", bufs=2))
        onepool = ctx.enter_context(tc.tile_pool(name="onepool", bufs=1))
        pspool = ctx.enter_context(tc.tile_pool(name="pspool", bufs=1, space="PSUM"))

        # ---- constants; weights ride the ACT DMA queue (idle at startup) so
        # they stream in parallel with the chunk-0 staging on the Sync queue
        # per-partition bias columns: [128, N_PAIR] with pair p in column p
        bq_sb = onepool.tile([128, N_PAIR], F32, name="bq_sb")
        nc.scalar.dma_start(bq_sb[:], bq_in.rearrange("(p d) one -> d (p one)", p=N_PAIR))
        bk_sb = onepool.tile([128, N_PAIR], F32, name="bk_sb")
        nc.scalar.dma_start(bk_sb[:], bk_in.rearrange("(p d) one -> d (p one)", p=N_PAIR))
        bv_sb = onepool.tile([1, DG], F32, name="bv_sb")
        nc.scalar.dma_start(bv_sb[:], bv_in[:])
        # broadcast V bias across partitions once (added during the V copy-out)
        bvb = onepool.tile([128, DG], F32, name="bvb")
        nc.gpsimd.partition_broadcast(bvb[:], bv_sb[:])
        mask_sb = mpool.tile([KB, 2, KB], BF16, name="mask_sb")
        nc.scalar.dma_start(mask_sb[:], mask_in[:])
        # PE warm-up during the DMA-bound prologue: throwaway matmuls take
        # the HAM clock gate to 8/8 before the first real matmul issues
        wa = onepool.tile([128, 128], BF16, name="wa")
        nc.vector.memset(wa[:], 0.0)
        wb = onepool.tile([128, SC], BF16, name="wb")
        nc.vector.memset(wb[:], 0.0)
        for _ in range(12):
            wps = pspool.tile([128, SC], F32, name="wps", tag="psa", bufs=2)
            nc.tensor.matmul(wps[:], wa[:], wb[:], start=True, stop=True)

        # ---- persistent data regions ----
        # KT: per (head-pair, s-chunk) tile [128, SC]; rows 0:64 head 2p.
        KT = [[ktpool.tile([128, SC], BF16, name=f"KT{p}_{sc}") for sc in range(N_SC)]
              for p in range(N_PAIR)]
        # V: per key-block tile [128, HG, 128]; per head 64 value cols + 64 ones
        # cols, so the PV matmul emits the softmax denominator replicated across
        # output partitions 64:128 (no cross-partition broadcast needed later).
        V = [vpool.tile([128, HG, 128], BF16, name=f"V{kb}") for kb in range(N_KB)]
        for kb in range(N_KB):
            nc.vector.memset(V[kb][:, :, 64:128], 1.0)

        w_sb = {}
        for nm, w_in in (("wq", wq_in), ("wk", wk_in), ("wv", wv_in)):
            w_sb[nm] = []
            for dm in range(N_DM):
                t = wpool.tile([128, DG], BF16, name=f"{nm}_{dm}", tag="w")
                nc.scalar.dma_start(t[:], w_in[dm * 128:(dm + 1) * 128, :])
                w_sb[nm].append(t)
        wo_sb = []
        for p in range(N_PAIR):
            t = wopool.tile([128, D], BF16, name=f"wo_{p}")
            nc.scalar.dma_start(t[:], wo_in[p * 128:(p + 1) * 128, :])
            wo_sb.append(t)

        def ps_small(name):
            return pspool.tile([128, SC], F32, name=name, tag="psa", bufs=2)

        def stage_chunk(nm, xT_in, sc):
            xs = []
            for dm in range(N_DM):
                t = stage.tile([128, SC], BF16, name=f"{nm}s{sc}_{dm}", tag="stage")
                nc.sync.dma_start(
                    t[:], xT_in[dm * 128:(dm + 1) * 128, sc * SC:(sc + 1) * SC]
                )
                xs.append(t)
            return xs

        def stage_all(sc):
            return {nm: stage_chunk(nm, xT, sc)
                    for nm, xT in (("q", qT_in), ("k", kT_in), ("v", vT_in))}

        # ---- fine-grained emission steps (for PE-filler interleaving) ----
        def proj_qk_steps(nm, xs, bias, dst, p):
            """Steps projecting head-pair p of a q/k chunk into dst [128, SC]."""
            hold = {}
            steps = []
            for dm in range(N_DM):
                def mid(dm=dm):
                    if dm == 0:
                        hold["ps"] = ps_small(f"ps_{nm}")
                    nc.tensor.matmul(
                        hold["ps"][:],
                        w_sb["w" + nm][dm][:, p * 128:(p + 1) * 128],
                        xs[dm][:],
                        start=(dm == 0), stop=(dm == N_DM - 1),
                    )
                steps.append(mid)

            def out():  # bias add folded into the PSUM->SBUF copy
                nc.vector.tensor_scalar_add(dst[:], hold["ps"][:],
                                            bias[:, p:p + 1])
            steps.append(out)
            return steps

        def proj_v_steps(xs, sb, kb):
            """Steps projecting 128-row value block kb into V[kb]."""
            hold = {}
            steps = []
            for dm in range(N_DM):
                def mid(dm=dm):
                    if dm == 0:
                        hold["ps"] = ps_small("ps_v")
                    nc.tensor.matmul(
                        hold["ps"][:],
                        xs[dm][:, sb * 128:(sb + 1) * 128],
                        w_sb["wv"][dm][:],
                        start=(dm == 0), stop=(dm == N_DM - 1),
                    )
                steps.append(mid)

            def out():  # bias add folded into the PSUM->SBUF copy
                nc.vector.tensor_tensor(
                    V[kb][:, :, 0:64],
                    hold["ps"][:].rearrange("p (h d) -> p h d", h=HG),
                    bvb[:].rearrange("p (h d) -> p h d", h=HG),
                    mybir.AluOpType.add,
                )
            steps.append(out)
            return steps

        def proj_chunk_steps(sc, xsmap, QTc):
            steps = []
            for p in range(N_PAIR):
                steps += proj_qk_steps("q", xsmap["q"], bq_sb, QTc[p], p)
            for p in range(N_PAIR):
                steps += proj_qk_steps("k", xsmap["k"], bk_sb, KT[p][sc], p)
            for sb in range(4):
                steps += proj_v_steps(xsmap["v"], sb, sc * 4 + sb)
            return steps

        def outproj_steps(qc, ctx_pairs):
            """Steps projecting chunk qc's normalized context to y[qc]."""
            steps = []
            for sb in range(4):
                hold = {}

                def mkyst(hold=hold):
                    hold["yst"] = ypool.tile([128, D], BF16, name="yst", tag="y")
                steps.append(mkyst)
                for dc in range(2):
                    for p in range(N_PAIR):
                        def mm(hold=hold, dc=dc, p=p, sb=sb):
                            if p == 0:
                                hold["yps"] = ps_small("yps")
                            nc.tensor.matmul(
                                hold["yps"][:],
                                ctx_pairs[p][:, sb * 128:(sb + 1) * 128],
                                wo_sb[p][:, dc * SC:(dc + 1) * SC],
                                start=(p == 0), stop=(p == N_PAIR - 1),
                            )
                        steps.append(mm)

                    def cp(hold=hold, dc=dc):
                        nc.vector.tensor_copy(
                            hold["yst"][:, dc * SC:(dc + 1) * SC], hold["yps"][:])
                    steps.append(cp)

                def store(hold=hold, sb=sb):
                    # interleave order puts all stage loads ahead of stores, so
                    # the Sync queue has no head-of-line risk
                    row = qc * SC + sb * 128
                    nc.sync.dma_start(y_out[row:row + 128, :], hold["yst"][:])
                steps.append(store)
            return steps

        # ---- attention for one q-chunk, with filler drip ----
        def attention(qc, QTc, filler, hold_back=0):
            kbmax = 4 * (qc + 1)
            blocks = [(p, kb) for p in range(N_PAIR) for kb in range(kbmax)]
            nb = len(blocks)
            drip_n = len(filler) - hold_back
            emitted = 0

            def drip(n):
                nonlocal emitted
                target = min(drip_n, (n + 1) * drip_n // nb)
                while emitted < target:
                    filler[emitted]()
                    emitted += 1

            scps_l = [None] * (nb + 1)
            ctx01 = {}
            ctx_pairs = [None] * N_PAIR

            def scores(n):
                p, kb = blocks[n]
                off = max(kb - 4 * qc, 0) * KB
                kt = KT[p][kb // 4]
                kcol = (kb % 4) * KB
                scps = pspool.tile([128, 2, SC], F32, name="scps", tag="pssc",
                                   bufs=2)
                nc.tensor.matmul(
                    scps[:, 0, off:SC], kt[0:64, kcol:kcol + KB],
                    QTc[p][0:64, off:SC], start=True, stop=True,
                )
                nc.tensor.matmul(
                    scps[:, 1, off:SC], kt[64:128, kcol:kcol + KB],
                    QTc[p][64:128, off:SC], start=True, stop=True,
                )
                scps_l[n] = scps

            scores(0)
            for n, (p, kb) in enumerate(blocks):
                j = kb - 4 * qc  # >=0: diagonal-crossing block
                off = max(j, 0) * KB  # live q-range is [off, SC)
                if kb == 0:
                    ctx01[p] = (
                        pspool.tile([128, SC], F32, name="ctx0", tag="psctx0",
                                    bufs=1),
                        pspool.tile([128, SC], F32, name="ctx1", tag="psctx1",
                                    bufs=1),
                    )
                scps = scps_l[n]
                e = epool.tile([128, 2, SC], BF16, name="e", tag="e", bufs=4)
                if off == 0:  # contiguous 2D view keeps ACT at full rate
                    nc.scalar.activation(
                        e[:].rearrange("p h s -> p (h s)"),
                        scps[:].rearrange("p h s -> p (h s)"),
                        mybir.ActivationFunctionType.Exp, scale=EXPSCALE,
                    )
                else:
                    nc.scalar.activation(
                        e[:, :, off:SC], scps[:, :, off:SC],
                        mybir.ActivationFunctionType.Exp, scale=EXPSCALE,
                    )
                if j >= 0:  # mask the leading 128-col triangle (both heads)
                    nc.vector.tensor_mul(e[:, :, off:off + KB],
                                         e[:, :, off:off + KB], mask_sb[:])
                # scores of the next block issue before this block's PV so the
                # PE isn't blocked on the exp, and ACT always has work queued
                if n + 1 < nb:
                    scores(n + 1)
                drip(n)
                first, last = kb == 0, kb == kbmax - 1
                ctx0, ctx1 = ctx01[p]
                nc.tensor.matmul(
                    ctx0[:, off:SC], V[kb][:, 2 * p, :], e[:, 0, off:SC],
                    start=first, stop=last,
                )
                nc.tensor.matmul(
                    ctx1[:, off:SC], V[kb][:, 2 * p + 1, :], e[:, 1, off:SC],
                    start=first, stop=last,
                )
                if last:
                    # normalize: ctx rows 0:64 / ctx row 64
                    cp = cpool.tile([128, SC], BF16, name="cp", tag="ctx")
                    final = qc == N_SC - 1 and p == N_PAIR - 1
                    for i, cps in ((0, ctx0), (1, ctx1)):
                        den = rpool.tile([64, SC], F32, name="den", tag="rec",
                                         bufs=4)
                        # reciprocal mis-reads PSUM on HW: hop via SBUF.
                        # For the very last pair the copy goes via ACT (idle by
                        # then) so the DVE chain that gates outproj is shorter.
                        if final:
                            nc.scalar.copy(den[:], cps[64:128, :])
                        else:
                            nc.vector.tensor_copy(den[:], cps[64:128, :])
                        rec = rpool.tile([64, SC], F32, name="rec", tag="rec",
                                         bufs=4)
                        nc.vector.reciprocal_approx_fast(rec[:], den[:])
                        nc.vector.tensor_tensor(
                            cp[i * 64:(i + 1) * 64, :], cps[0:64, :], rec[:],
                            mybir.AluOpType.mult,
                        )
                    ctx_pairs[p] = cp
            while emitted < len(filler):
                filler[emitted]()
                emitted += 1
            return ctx_pairs

        # ---- main pipeline ----
        stages = [stage_all(0), stage_all(1), None, None]
        QTcs = [[qtpool.tile([128, SC], BF16, name=f"QT{p}_{sc}", tag="qtc")
                 for p in range(N_PAIR)] for sc in range(N_SC)]
        # chunk-0 projections run standalone (nothing to overlap with yet)
        for st in proj_chunk_steps(0, stages[0], QTcs[0]):
            st()
        ctxs = [None] * N_SC
        for qc in range(N_SC):
            if qc + 2 < N_SC:
                stages[qc + 2] = stage_all(qc + 2)
            filler = []
            if qc == 1:
                filler += outproj_steps(0, ctxs[0])
            elif qc == 3:
                filler += outproj_steps(1, ctxs[1])
                filler += outproj_steps(2, ctxs[2])
            if qc + 1 < N_SC:
                filler += proj_chunk_steps(qc + 1, stages[qc + 1],
                                           QTcs[qc + 1])
            # hold back the tail of the last chunk's filler: it is emitted
            # after the final pair's norm, so the PE has work during the norm's
            # DVE chain and the HAM clock gate stays at 8/8 into the epilogue
            ctxs[qc] = attention(qc, QTcs[qc], filler,
                                 hold_back=12 if qc == N_SC - 1 else 0)
        for st in outproj_steps(N_SC - 1, ctxs[N_SC - 1]):
            st()

    nc.compile()
    return nc


def make_inputs(q, k, v, wq, bq, wk, bk, wv, bv, wo):
    """Host-side shard + layout prep. Returns list of 8 per-core input dicts."""
    qj = np.arange(KB)[None, :]
    ki = np.arange(KB)[:, None]
    mask = np.ascontiguousarray(
        np.repeat((qj >= ki).astype(NP_BF16)[:, None, :], 2, axis=1))

    def bt(a):  # bf16 contiguous
        return np.ascontiguousarray(np.asarray(a).astype(NP_BF16))

    qT = [bt(np.asarray(q[b]).T) for b in range(B)]
    kT = [bt(np.asarray(k[b]).T) for b in range(B)]
    vT = [bt(np.asarray(v[b]).T) for b in range(B)]

    in_maps = []
    for c in range(8):
        b, g = c // 2, c % 2
        sl = slice(g * DG, (g + 1) * DG)
        in_maps.append({
            "qT": qT[b], "kT": kT[b], "vT": vT[b],
            "wq": bt(wq[:, sl]),
            "wk": bt(wk[:, sl]),
            "wv": bt(wv[:, sl]),
            "wo": bt(wo[sl, :]),
            "bqT": np.ascontiguousarray(np.asarray(bq[sl], np.float32)).reshape(DG, 1),
            "bkT": np.ascontiguousarray(np.asarray(bk[sl], np.float32)).reshape(DG, 1),
            "bv": np.ascontiguousarray(np.asarray(bv[sl], np.float32)).reshape(1, DG),
            "masks": mask,
        })
    return in_maps


def combine_outputs(results, bo):
    """Sum the two row-parallel partials per batch and add the output bias."""
    out = np.empty((B, S, D), np.float32)
    for b in range(B):
        out[b] = (results[2 * b]["y"].astype(np.float32)
                  + results[2 * b + 1]["y"].astype(np.float32)
                  + np.asarray(bo, np.float32)[None, :])
    return out


_NC_CACHE = {}


def kernel(x, q, k, v, mask, wq, bq, wk, bk, wv, bv, wo, bo):
    # x is unused (overwritten in the reference forward); mask is the causal
    # tril mask, which is hardcoded in the on-device masking.
    if "nc" not in _NC_CACHE:
        _NC_CACHE["nc"] = build_program()
    nc = _NC_CACHE["nc"]
    in_maps = make_inputs(q, k, v, wq, bq, wk, bk, wv, bv, wo)
    out = None
    try:
        r = run_bass_kernel_spmd(nc, in_maps, core_ids=list(range(8)))
        out = combine_outputs(r.results, bo)
    except Exception:
        pass
    if out is None or not np.isfinite(out).all():
        # defensive: retry once on a transient exec failure or bad readback
        r = run_bass_kernel_spmd(nc, in_maps, core_ids=list(range(8)))
        out = combine_outputs(r.results, bo)
    return out


# revision 19
# speedup vs baseline: 1.0072x; 1.0072x over previous
"""Multi-head attention (B=4, S=2048, D=1024, H=16, causal) on 8 TRN2 NeuronCores.

Sharding: batch (4) x head-group (2 groups of 8 heads) = 8 cores.
Megatron-style: wq/wk/wv column-parallel, wo row-parallel; the 2-way partial-sum
of the row-parallel output projection is folded into the host-side unshard.

Per-core algorithm (heads h in the core's group, q-chunks of 512 queries):
  QT[dk, s], KT[dk, s] = (x @ w + b)^T via PE matmuls on host-pre-transposed
  inputs; V[s, dv] likewise, with 64 ones-columns appended per head so that
  the PV matmul also produces softmax denominators.
  scoresT[k, q] = KT-slices x QT (two heads packed in the 128-partition dim,
  concurrent via PE row tiling since dk=64).
  E = exp(scoresT/8) on ACT (no max-subtraction needed: scores ~ N(0,1)).
  Causality: fully-masked key-blocks are never computed; diagonal-crossing
  blocks are trapezoid-sliced to their live q-range and only the leading
  128-column triangle gets a mask multiply.
  ctxT[dv, q] accumulates V-slices x E in PSUM; rows 64:128 = sum(E).
  The 64 ones-columns make the PV matmul emit the softmax denominator
  replicated across output partitions 64:128, so normalization is a pure
  DVE chain: copy -> approx-reciprocal -> multiply.
  y_partial[s, do] = sum over head-pairs of ctxT-slices x wo-rows (PSUM accum).

All matmul operands are bf16 (accumulation stays fp32 in PSUM); softmax
denominators, reciprocals and the final output stay fp32.

Scheduling: engines execute their instruction streams in order, so emission
order is the schedule. Attention is software-pipelined (scores of block n+1
issue before the PV of block n, hiding the ACT exp latency), and the next
chunk's projection + previous chunk's output-projection matmuls are dripped
between attention blocks as PE filler so the tensor engine never idles long
enough for the HAM clock gate to re-throttle it to 1.2 GHz.
"""
import sys
import numpy as np
import ml_dtypes

sys.path.insert(0, "/opt/trn_rl_repo")

from contextlib import ExitStack

import concourse.bacc as bacc
import concourse.tile as tile
from concourse import mybir
from concourse.bass_utils import run_bass_kernel_spmd

F32 = mybir.dt.float32
BF16 = mybir.dt.bfloat16
NP_BF16 = ml_dtypes.bfloat16

B, S, D, H = 4, 2048, 1024, 16
DK = D // H          # 64
HG = H // 2          # 8 heads per core
DG = HG * DK         # 512 columns per core group
SC = 512             # query-chunk width
KB = 128             # key-block height
N_SC = S // SC       # 4
N_KB = S // KB       # 16
N_DM = D // 128      # 8 contraction tiles for projections
N_PAIR = HG // 2     # 4 head pairs per core
EXPSCALE = 1.0 / 8.0  # 1/sqrt(DK)


def build_program():
    """Emit the SPMD Bass program (identical on all 8 cores)."""
    nc = bacc.Bacc("TRN2", target_bir_lowering=False, debug=False)

    qT_in = nc.dram_tensor("qT", [D, S], BF16, kind="ExternalInput").ap()
    kT_in = nc.dram_tensor("kT", [D, S], BF16, kind="ExternalInput").ap()
    vT_in = nc.dram_tensor("vT", [D, S], BF16, kind="ExternalInput").ap()
    wq_in = nc.dram_tensor("wq", [D, DG], BF16, kind="ExternalInput").ap()
    wk_in = nc.dram_tensor("wk", [D, DG], BF16, kind="ExternalInput").ap()
    wv_in = nc.dram_tensor("wv", [D, DG], BF16, kind="ExternalInput").ap()
    wo_in = nc.dram_tensor("wo", [DG, D], BF16, kind="ExternalInput").ap()
    bq_in = nc.dram_tensor("bqT", [DG, 1], F32, kind="ExternalInput").ap()
    bk_in = nc.dram_tensor("bkT", [DG, 1], F32, kind="ExternalInput").ap()
    bv_in = nc.dram_tensor("bv", [1, DG], F32, kind="ExternalInput").ap()
    # leading-triangle causal mask: mask[ki, h, qj] = (qj >= ki), [128, 2, 128]
    mask_in = nc.dram_tensor("masks", [KB, 2, KB], BF16, kind="ExternalInput").ap()
    y_out = nc.dram_tensor("y", [S, D], BF16, kind="ExternalOutput").ap()

    with tile.TileContext(nc) as tc, ExitStack() as ctx:
        stage = ctx.enter_context(tc.tile_pool(name="stage", bufs=56))
        wpool = ctx.enter_context(tc.tile_pool(name="wpool", bufs=24))
        wopool = ctx.enter_context(tc.tile_pool(name="wopool", bufs=1))
        qtpool = ctx.enter_context(tc.tile_pool(name="qtpool", bufs=8))
        ktpool = ctx.enter_context(tc.tile_pool(name="ktpool", bufs=1))
        vpool = ctx.enter_context(tc.tile_pool(name="vpool", bufs=1))
        epool = ctx.enter_context(tc.tile_pool(name="epool", bufs=4))
        cpool = ctx.enter_context(tc.tile_pool(name="cpool", bufs=12))
        mpool = ctx.enter_context(tc.tile_pool(name="mpool", bufs=1))
        ypool = ctx.enter_context(tc.tile_pool(name="ypool", bufs=2))
        rpool = ctx.enter_context(tc.tile_pool(name="rpool", bufs=2))
        onepool = ctx.enter_context(tc.tile_pool(name="onepool", bufs=1))
        pspool = ctx.enter_context(tc.tile_pool(name="pspool", bufs=1, space="PSUM"))

        # ---- constants; weights ride the ACT DMA queue (idle at startup) so
        # they stream in parallel with the chunk-0 staging on the Sync queue
        # per-partition bias columns: [128, N_PAIR] with pair p in column p
        bq_sb = onepool.tile([128, N_PAIR], F32, name="bq_sb")
        nc.scalar.dma_start(bq_sb[:], bq_in.rearrange("(p d) one -> d (p one)", p=N_PAIR))
        bk_sb = onepool.tile([128, N_PAIR], F32, name="bk_sb")
        nc.scalar.dma_start(bk_sb[:], bk_in.rearrange("(p d) one -> d (p one)", p=N_PAIR))
        bv_sb = onepool.tile([1, DG], F32, name="bv_sb")
        nc.scalar.dma_start(bv_sb[:], bv_in[:])
        # broadcast V bias across partitions once (added during the V copy-out)
        bvb = onepool.tile([128, DG], F32, name="bvb")
        nc.gpsimd.partition_broadcast(bvb[:], bv_sb[:])
        mask_sb = mpool.tile([KB, 2, KB], BF16, name="mask_sb")
        nc.scalar.dma_start(mask_sb[:], mask_in[:])
        # PE warm-up during the DMA-bound prologue: throwaway matmuls take
        # the HAM clock gate to 8/8 before the first real matmul issues
        wa = onepool.tile([128, 128], BF16, name="wa")
        nc.vector.memset(wa[:], 0.0)
        wb = onepool.tile([128, SC], BF16, name="wb")
        nc.vector.memset(wb[:], 0.0)
        for _ in range(12):
            wps = pspool.tile([128, SC], F32, name="wps", tag="psa", bufs=2)
            nc.tensor.matmul(wps[:], wa[:], wb[:], start=True, stop=True)

        # ---- persistent data regions ----
        # KT: per (head-pair, s-chunk) tile [128, SC]; rows 0:64 head 2p.
        KT = [[ktpool.tile([128, SC], BF16, name=f"KT{p}_{sc}") for sc in range(N_SC)]
              for p in range(N_PAIR)]
        # V: per key-block tile [128, HG, 128]; per head 64 value cols + 64 ones
        # cols, so the PV matmul emits the softmax denominator replicated across
        # output partitions 64:128 (no cross-partition broadcast needed later).
        V = [vpool.tile([128, HG, 128], BF16, name=f"V{kb}") for kb in range(N_KB)]
        for kb in range(N_KB):
            # GPSIMD (idle at startup): on DVE these 16 memsets would queue
            # ahead of the first projection copy-outs and starve the PE of
            # PSUM buffers
            nc.gpsimd.memset(V[kb][:, :, 64:128], 1.0)

        w_sb = {}
        for nm, w_in in (("wq", wq_in), ("wk", wk_in), ("wv", wv_in)):
            w_sb[nm] = []
            for dm in range(N_DM):
                t = wpool.tile([128, DG], BF16, name=f"{nm}_{dm}", tag="w")
                nc.scalar.dma_start(t[:], w_in[dm * 128:(dm + 1) * 128, :])
                w_sb[nm].append(t)
        wo_sb = []
        for p in range(N_PAIR):
            t = wopool.tile([128, D], BF16, name=f"wo_{p}")
            nc.scalar.dma_start(t[:], wo_in[p * 128:(p + 1) * 128, :])
            wo_sb.append(t)

        def ps_small(name):
            return pspool.tile([128, SC], F32, name=name, tag="psa", bufs=2)

        def stage_chunk(nm, xT_in, sc):
            xs = []
            for dm in range(N_DM):
                t = stage.tile([128, SC], BF16, name=f"{nm}s{sc}_{dm}", tag="stage")
                nc.sync.dma_start(
                    t[:], xT_in[dm * 128:(dm + 1) * 128, sc * SC:(sc + 1) * SC]
                )
                xs.append(t)
            return xs

        def stage_all(sc):
            return {nm: stage_chunk(nm, xT, sc)
                    for nm, xT in (("q", qT_in), ("k", kT_in), ("v", vT_in))}

        # ---- fine-grained emission steps (for PE-filler interleaving) ----
        def proj_qk_steps(nm, xs, bias, dst, p):
            """Steps projecting head-pair p of a q/k chunk into dst [128, SC]."""
            hold = {}
            steps = []
            for dm in range(N_DM):
                def mid(dm=dm):
                    if dm == 0:
                        hold["ps"] = ps_small(f"ps_{nm}")
                    nc.tensor.matmul(
                        hold["ps"][:],
                        w_sb["w" + nm][dm][:, p * 128:(p + 1) * 128],
                        xs[dm][:],
                        start=(dm == 0), stop=(dm == N_DM - 1),
                    )
                steps.append(mid)

            def out():  # bias add folded into the PSUM->SBUF copy
                nc.vector.tensor_scalar_add(dst[:], hold["ps"][:],
                                            bias[:, p:p + 1])
            steps.append(out)
            return steps

        def proj_v_steps(xs, sb, kb):
            """Steps projecting 128-row value block kb into V[kb]."""
            hold = {}
            steps = []
            for dm in range(N_DM):
                def mid(dm=dm):
                    if dm == 0:
                        hold["ps"] = ps_small("ps_v")
                    nc.tensor.matmul(
                        hold["ps"][:],
                        xs[dm][:, sb * 128:(sb + 1) * 128],
                        w_sb["wv"][dm][:],
                        start=(dm == 0), stop=(dm == N_DM - 1),
                    )
                steps.append(mid)

            def out():  # bias add folded into the PSUM->SBUF copy
                nc.vector.tensor_tensor(
                    V[kb][:, :, 0:64],
                    hold["ps"][:].rearrange("p (h d) -> p h d", h=HG),
                    bvb[:].rearrange("p (h d) -> p h d", h=HG),
                    mybir.AluOpType.add,
                )
            steps.append(out)
            return steps

        def proj_chunk_steps(sc, xsmap, QTc):
            steps = []
            for p in range(N_PAIR):
                steps += proj_qk_steps("q", xsmap["q"], bq_sb, QTc[p], p)
            for p in range(N_PAIR):
                steps += proj_qk_steps("k", xsmap["k"], bk_sb, KT[p][sc], p)
            for sb in range(4):
                steps += proj_v_steps(xsmap["v"], sb, sc * 4 + sb)
            return steps

        def outproj_steps(qc, ctx_pairs):
            """Steps projecting chunk qc's normalized context to y[qc]."""
            steps = []
            for sb in range(4):
                hold = {}

                def mkyst(hold=hold):
                    hold["yst"] = ypool.tile([128, D], BF16, name="yst", tag="y")
                steps.append(mkyst)
                for dc in range(2):
                    for p in range(N_PAIR):
                        def mm(hold=hold, dc=dc, p=p, sb=sb):
                            if p == 0:
                                hold["yps"] = ps_small("yps")
                            nc.tensor.matmul(
                                hold["yps"][:],
                                ctx_pairs[p][:, sb * 128:(sb + 1) * 128],
                                wo_sb[p][:, dc * SC:(dc + 1) * SC],
                                start=(p == 0), stop=(p == N_PAIR - 1),
                            )
                        steps.append(mm)

                    def cp(hold=hold, dc=dc):
                        nc.vector.tensor_copy(
                            hold["yst"][:, dc * SC:(dc + 1) * SC], hold["yps"][:])
                    steps.append(cp)

                def store(hold=hold, sb=sb):
                    # interleave order puts all stage loads ahead of stores, so
                    # the Sync queue has no head-of-line risk
                    row = qc * SC + sb * 128
                    nc.sync.dma_start(y_out[row:row + 128, :], hold["yst"][:])
                steps.append(store)
            return steps

        # ---- attention for one q-chunk, with filler drip ----
        def attention(qc, QTc, filler, hold_back=0):
            kbmax = 4 * (qc + 1)
            blocks = [(p, kb) for p in range(N_PAIR) for kb in range(kbmax)]
            nb = len(blocks)
            drip_n = len(filler) - hold_back
            emitted = 0

            def drip(n):
                nonlocal emitted
                target = min(drip_n, (n + 1) * drip_n // nb)
                while emitted < target:
                    filler[emitted]()
                    emitted += 1

            scps_l = [None] * (nb + 1)
            ctx01 = {}
            ctx_pairs = [None] * N_PAIR

            def scores(n):
                p, kb = blocks[n]
                off = max(kb - 4 * qc, 0) * KB
                kt = KT[p][kb // 4]
                kcol = (kb % 4) * KB
                scps = pspool.tile([128, 2, SC], F32, name="scps", tag="pssc",
                                   bufs=2)
                nc.tensor.matmul(
                    scps[:, 0, off:SC], kt[0:64, kcol:kcol + KB],
                    QTc[p][0:64, off:SC], start=True, stop=True,
                )
                nc.tensor.matmul(
                    scps[:, 1, off:SC], kt[64:128, kcol:kcol + KB],
                    QTc[p][64:128, off:SC], start=True, stop=True,
                )
                scps_l[n] = scps

            scores(0)
            for n, (p, kb) in enumerate(blocks):
                j = kb - 4 * qc  # >=0: diagonal-crossing block
                off = max(j, 0) * KB  # live q-range is [off, SC)
                if kb == 0:
                    ctx01[p] = (
                        pspool.tile([128, SC], F32, name="ctx0", tag="psctx0",
                                    bufs=1),
                        pspool.tile([128, SC], F32, name="ctx1", tag="psctx1",
                                    bufs=1),
                    )
                scps = scps_l[n]
                e = epool.tile([128, 2, SC], BF16, name="e", tag="e", bufs=4)
                if off == 0:  # contiguous 2D view keeps ACT at full rate
                    nc.scalar.activation(
                        e[:].rearrange("p h s -> p (h s)"),
                        scps[:].rearrange("p h s -> p (h s)"),
                        mybir.ActivationFunctionType.Exp, scale=EXPSCALE,
                    )
                else:
                    nc.scalar.activation(
                        e[:, :, off:SC], scps[:, :, off:SC],
                        mybir.ActivationFunctionType.Exp, scale=EXPSCALE,
                    )
                if j >= 0:  # mask the leading 128-col triangle (both heads)
                    nc.vector.tensor_mul(e[:, :, off:off + KB],
                                         e[:, :, off:off + KB], mask_sb[:])
                # scores of the next block issue before this block's PV so the
                # PE isn't blocked on the exp, and ACT always has work queued
                if n + 1 < nb:
                    scores(n + 1)
                drip(n)
                first, last = kb == 0, kb == kbmax - 1
                ctx0, ctx1 = ctx01[p]
                nc.tensor.matmul(
                    ctx0[:, off:SC], V[kb][:, 2 * p, :], e[:, 0, off:SC],
                    start=first, stop=last,
                )
                nc.tensor.matmul(
                    ctx1[:, off:SC], V[kb][:, 2 * p + 1, :], e[:, 1, off:SC],
                    start=first, stop=last,
                )
                if last:
                    # normalize: ctx rows 0:64 / ctx row 64
                    cp = cpool.tile([128, SC], BF16, name="cp", tag="ctx")
                    final = qc == N_SC - 1 and p == N_PAIR - 1
                    for i, cps in ((0, ctx0), (1, ctx1)):
                        den = rpool.tile([64, SC], F32, name="den", tag="rec",
                                         bufs=4)
                        # reciprocal mis-reads PSUM on HW: hop via SBUF.
                        # For the very last pair the copy goes via ACT (idle by
                        # then) so the DVE chain that gates outproj is shorter.
                        if final:
                            nc.scalar.copy(den[:], cps[64:128, :])
                        else:
                            nc.vector.tensor_copy(den[:], cps[64:128, :])
                        rec = rpool.tile([64, SC], F32, name="rec", tag="rec",
                                         bufs=4)
                        nc.vector.reciprocal_approx_fast(rec[:], den[:])
                        nc.vector.tensor_tensor(
                            cp[i * 64:(i + 1) * 64, :], cps[0:64, :], rec[:],
                            mybir.AluOpType.mult,
                        )
                    ctx_pairs[p] = cp
            while emitted < len(filler):
                filler[emitted]()
                emitted += 1
            return ctx_pairs

        # ---- main pipeline ----
        stages = [stage_all(0), stage_all(1), None, None]
        QTcs = [[qtpool.tile([128, SC], BF16, name=f"QT{p}_{sc}", tag="qtc")
                 for p in range(N_PAIR)] for sc in range(N_SC)]
        # chunk-0 projections run standalone (nothing to overlap with yet)
        for st in proj_chunk_steps(0, stages[0], QTcs[0]):
            st()
        ctxs = [None] * N_SC
        for qc in range(N_SC):
            if qc + 2 < N_SC:
                stages[qc + 2] = stage_all(qc + 2)
            filler = []
            if qc == 1:
                filler += outproj_steps(0, ctxs[0])
            elif qc == 3:
                filler += outproj_steps(1, ctxs[1])
                filler += outproj_steps(2, ctxs[2])
            if qc + 1 < N_SC:
                filler += proj_chunk_steps(qc + 1, stages[qc + 1],
                                           QTcs[qc + 1])
            # hold back the tail of the last chunk's filler: it is emitted
            # after the final pair's norm, so the PE has work during the norm's
            # DVE chain and the HAM clock gate stays at 8/8 into the epilogue
            ctxs[qc] = attention(qc, QTcs[qc], filler,
                                 hold_back=24 if qc == N_SC - 1 else 0)
        for st in outproj_steps(N_SC - 1, ctxs[N_SC - 1]):
            st()

    nc.compile()
    return nc


def make_inputs(q, k, v, wq, bq, wk, bk, wv, bv, wo):
    """Host-side shard + layout prep. Returns list of 8 per-core input dicts."""
    qj = np.arange(KB)[None, :]
    ki = np.arange(KB)[:, None]
    mask = np.ascontiguousarray(
        np.repeat((qj >= ki).astype(NP_BF16)[:, None, :], 2, axis=1))

    def bt(a):  # bf16 contiguous
        return np.ascontiguousarray(np.asarray(a).astype(NP_BF16))

    qT = [bt(np.asarray(q[b]).T) for b in range(B)]
    kT = [bt(np.asarray(k[b]).T) for b in range(B)]
    vT = [bt(np.asarray(v[b]).T) for b in range(B)]

    in_maps = []
    for c in range(8):
        b, g = c // 2, c % 2
        sl = slice(g * DG, (g + 1) * DG)
        in_maps.append({
            "qT": qT[b], "kT": kT[b], "vT": vT[b],
            "wq": bt(wq[:, sl]),
            "wk": bt(wk[:, sl]),
            "wv": bt(wv[:, sl]),
            "wo": bt(wo[sl, :]),
            "bqT": np.ascontiguousarray(np.asarray(bq[sl], np.float32)).reshape(DG, 1),
            "bkT": np.ascontiguousarray(np.asarray(bk[sl], np.float32)).reshape(DG, 1),
            "bv": np.ascontiguousarray(np.asarray(bv[sl], np.float32)).reshape(1, DG),
            "masks": mask,
        })
    return in_maps


def combine_outputs(results, bo):
    """Sum the two row-parallel partials per batch and add the output bias."""
    out = np.empty((B, S, D), np.float32)
    for b in range(B):
        out[b] = (results[2 * b]["y"].astype(np.float32)
                  + results[2 * b + 1]["y"].astype(np.float32)
                  + np.asarray(bo, np.float32)[None, :])
    return out


_NC_CACHE = {}


def kernel(x, q, k, v, mask, wq, bq, wk, bk, wv, bv, wo, bo):
    # x is unused (overwritten in the reference forward); mask is the causal
    # tril mask, which is hardcoded in the on-device masking.
    if "nc" not in _NC_CACHE:
        _NC_CACHE["nc"] = build_program()
    nc = _NC_CACHE["nc"]
    in_maps = make_inputs(q, k, v, wq, bq, wk, bk, wv, bv, wo)
    out = None
    try:
        r = run_bass_kernel_spmd(nc, in_maps, core_ids=list(range(8)))
        out = combine_outputs(r.results, bo)
    except Exception:
        pass
    if out is None or not np.isfinite(out).all():
        # defensive: retry once on a transient exec failure or bad readback
        r = run_bass_kernel_spmd(nc, in_maps, core_ids=list(range(8)))
        out = combine_outputs(r.results, bo)
    return out


# revision 20
# speedup vs baseline: 1.0240x; 1.0167x over previous
"""Multi-head attention (B=4, S=2048, D=1024, H=16, causal) on 8 TRN2 NeuronCores.

Sharding: batch (4) x head-group (2 groups of 8 heads) = 8 cores.
Megatron-style: wq/wk/wv column-parallel, wo row-parallel; the 2-way partial-sum
of the row-parallel output projection is folded into the host-side unshard.

Per-core algorithm (heads h in the core's group, q-chunks of 512 queries):
  QT[dk, s], KT[dk, s] = (x @ w + b)^T via PE matmuls on host-pre-transposed
  inputs; V[s, dv] likewise, with 64 ones-columns appended per head so that
  the PV matmul also produces softmax denominators.
  scoresT[k, q] = KT-slices x QT (two heads packed in the 128-partition dim,
  concurrent via PE row tiling since dk=64).
  E = exp(scoresT/8) on ACT (no max-subtraction needed: scores ~ N(0,1)).
  Causality: fully-masked key-blocks are never computed; diagonal-crossing
  blocks are trapezoid-sliced to their live q-range and only the leading
  128-column triangle gets a mask multiply.
  ctxT[dv, q] accumulates V-slices x E in PSUM; rows 64:128 = sum(E).
  The 64 ones-columns make the PV matmul emit the softmax denominator
  replicated across output partitions 64:128, so normalization is a pure
  DVE chain: copy -> approx-reciprocal -> multiply.
  y_partial[s, do] = sum over head-pairs of ctxT-slices x wo-rows (PSUM accum).

All matmul operands are bf16 (accumulation stays fp32 in PSUM); softmax
denominators, reciprocals and the final output stay fp32.

Scheduling: engines execute their instruction streams in order, so emission
order is the schedule. Attention is software-pipelined (scores of block n+1
issue before the PV of block n, hiding the ACT exp latency), and the next
chunk's projection + previous chunk's output-projection matmuls are dripped
between attention blocks as PE filler so the tensor engine never idles long
enough for the HAM clock gate to re-throttle it to 1.2 GHz.
"""
import sys
import numpy as np
import ml_dtypes

sys.path.insert(0, "/opt/trn_rl_repo")

from contextlib import ExitStack

import concourse.bacc as bacc
import concourse.tile as tile
from concourse import mybir
from concourse.bass_utils import run_bass_kernel_spmd

F32 = mybir.dt.float32
BF16 = mybir.dt.bfloat16
NP_BF16 = ml_dtypes.bfloat16

B, S, D, H = 4, 2048, 1024, 16
DK = D // H          # 64
HG = H // 2          # 8 heads per core
DG = HG * DK         # 512 columns per core group
SC = 512             # query-chunk width
KB = 128             # key-block height
N_SC = S // SC       # 4
N_KB = S // KB       # 16
N_DM = D // 128      # 8 contraction tiles for projections
N_PAIR = HG // 2     # 4 head pairs per core
EXPSCALE = 1.0 / 8.0  # 1/sqrt(DK)


def build_program():
    """Emit the SPMD Bass program (identical on all 8 cores)."""
    nc = bacc.Bacc("TRN2", target_bir_lowering=False, debug=False)

    qT_in = nc.dram_tensor("qT", [D, S], BF16, kind="ExternalInput").ap()
    kT_in = nc.dram_tensor("kT", [D, S], BF16, kind="ExternalInput").ap()
    vT_in = nc.dram_tensor("vT", [D, S], BF16, kind="ExternalInput").ap()
    wq_in = nc.dram_tensor("wq", [D, DG], BF16, kind="ExternalInput").ap()
    wk_in = nc.dram_tensor("wk", [D, DG], BF16, kind="ExternalInput").ap()
    wv_in = nc.dram_tensor("wv", [D, DG], BF16, kind="ExternalInput").ap()
    wo_in = nc.dram_tensor("wo", [DG, D], BF16, kind="ExternalInput").ap()
    bq_in = nc.dram_tensor("bqT", [DG, 1], F32, kind="ExternalInput").ap()
    bk_in = nc.dram_tensor("bkT", [DG, 1], F32, kind="ExternalInput").ap()
    bv_in = nc.dram_tensor("bv", [1, DG], F32, kind="ExternalInput").ap()
    # leading-triangle causal mask: mask[ki, h, qj] = (qj >= ki), [128, 2, 128]
    mask_in = nc.dram_tensor("masks", [KB, 2, KB], BF16, kind="ExternalInput").ap()
    y_out = nc.dram_tensor("y", [S, D], BF16, kind="ExternalOutput").ap()

    with tile.TileContext(nc) as tc, ExitStack() as ctx:
        stage = ctx.enter_context(tc.tile_pool(name="stage", bufs=56))
        wpool = ctx.enter_context(tc.tile_pool(name="wpool", bufs=24))
        wopool = ctx.enter_context(tc.tile_pool(name="wopool", bufs=1))
        qtpool = ctx.enter_context(tc.tile_pool(name="qtpool", bufs=8))
        ktpool = ctx.enter_context(tc.tile_pool(name="ktpool", bufs=1))
        vpool = ctx.enter_context(tc.tile_pool(name="vpool", bufs=1))
        epool = ctx.enter_context(tc.tile_pool(name="epool", bufs=4))
        cpool = ctx.enter_context(tc.tile_pool(name="cpool", bufs=12))
        mpool = ctx.enter_context(tc.tile_pool(name="mpool", bufs=1))
        ypool = ctx.enter_context(tc.tile_pool(name="ypool", bufs=2))
        rpool = ctx.enter_context(tc.tile_pool(name="rpool", bufs=2))
        onepool = ctx.enter_context(tc.tile_pool(name="onepool", bufs=1))
        pspool = ctx.enter_context(tc.tile_pool(name="pspool", bufs=1, space="PSUM"))

        # ---- constants; weights ride the ACT DMA queue (idle at startup) so
        # they stream in parallel with the chunk-0 staging on the Sync queue
        # per-partition bias columns: [128, N_PAIR] with pair p in column p
        bq_sb = onepool.tile([128, N_PAIR], F32, name="bq_sb")
        nc.scalar.dma_start(bq_sb[:], bq_in.rearrange("(p d) one -> d (p one)", p=N_PAIR))
        bk_sb = onepool.tile([128, N_PAIR], F32, name="bk_sb")
        nc.scalar.dma_start(bk_sb[:], bk_in.rearrange("(p d) one -> d (p one)", p=N_PAIR))
        bv_sb = onepool.tile([1, DG], F32, name="bv_sb")
        nc.scalar.dma_start(bv_sb[:], bv_in[:])
        # broadcast V bias across partitions once (added during the V copy-out)
        bvb = onepool.tile([128, DG], F32, name="bvb")
        nc.gpsimd.partition_broadcast(bvb[:], bv_sb[:])
        mask_sb = mpool.tile([KB, 2, KB], BF16, name="mask_sb")
        nc.scalar.dma_start(mask_sb[:], mask_in[:])
        # PE warm-up during the DMA-bound prologue: throwaway matmuls take
        # the HAM clock gate to 8/8 before the first real matmul issues
        wa = onepool.tile([128, 128], BF16, name="wa")
        nc.vector.memset(wa[:], 0.0)
        wb = onepool.tile([128, SC], BF16, name="wb")
        nc.vector.memset(wb[:], 0.0)
        for _ in range(12):
            wps = pspool.tile([128, SC], F32, name="wps", tag="psa", bufs=2)
            nc.tensor.matmul(wps[:], wa[:], wb[:], start=True, stop=True)

        # ---- persistent data regions ----
        # KT: per (head-pair, s-chunk) tile [128, SC]; rows 0:64 head 2p.
        KT = [[ktpool.tile([128, SC], BF16, name=f"KT{p}_{sc}") for sc in range(N_SC)]
              for p in range(N_PAIR)]
        # V: per key-block tile [128, HG, 128]; per head 64 value cols + 64 ones
        # cols, so the PV matmul emits the softmax denominator replicated across
        # output partitions 64:128 (no cross-partition broadcast needed later).
        V = [vpool.tile([128, HG, 128], BF16, name=f"V{kb}") for kb in range(N_KB)]
        for kb in range(N_KB):
            # GPSIMD (idle at startup): on DVE these 16 memsets would queue
            # ahead of the first projection copy-outs and starve the PE of
            # PSUM buffers
            nc.gpsimd.memset(V[kb][:, :, 64:128], 1.0)

        w_sb = {}
        for nm, w_in in (("wq", wq_in), ("wk", wk_in), ("wv", wv_in)):
            w_sb[nm] = []
            for dm in range(N_DM):
                t = wpool.tile([128, DG], BF16, name=f"{nm}_{dm}", tag="w")
                nc.scalar.dma_start(t[:], w_in[dm * 128:(dm + 1) * 128, :])
                w_sb[nm].append(t)
        wo_sb = []
        for p in range(N_PAIR):
            t = wopool.tile([128, D], BF16, name=f"wo_{p}")
            nc.scalar.dma_start(t[:], wo_in[p * 128:(p + 1) * 128, :])
            wo_sb.append(t)

        def ps_small(name):
            return pspool.tile([128, SC], F32, name=name, tag="psa", bufs=2)

        def stage_chunk(nm, xT_in, sc):
            xs = []
            for dm in range(N_DM):
                t = stage.tile([128, SC], BF16, name=f"{nm}s{sc}_{dm}", tag="stage")
                nc.sync.dma_start(
                    t[:], xT_in[dm * 128:(dm + 1) * 128, sc * SC:(sc + 1) * SC]
                )
                xs.append(t)
            return xs

        def stage_all(sc):
            return {nm: stage_chunk(nm, xT, sc)
                    for nm, xT in (("q", qT_in), ("k", kT_in), ("v", vT_in))}

        # ---- fine-grained emission steps (for PE-filler interleaving) ----
        def proj_qk_steps(nm, xs, bias, dst, p):
            """Steps projecting head-pair p of a q/k chunk into dst [128, SC]."""
            hold = {}
            steps = []
            for dm in range(N_DM):
                def mid(dm=dm):
                    if dm == 0:
                        hold["ps"] = ps_small(f"ps_{nm}")
                    nc.tensor.matmul(
                        hold["ps"][:],
                        w_sb["w" + nm][dm][:, p * 128:(p + 1) * 128],
                        xs[dm][:],
                        start=(dm == 0), stop=(dm == N_DM - 1),
                    )
                steps.append(mid)

            def out():  # bias add folded into the PSUM->SBUF copy
                nc.vector.tensor_scalar_add(dst[:], hold["ps"][:],
                                            bias[:, p:p + 1])
            steps.append(out)
            return steps

        def proj_v_steps(xs, sb, kb):
            """Steps projecting 128-row value block kb into V[kb]."""
            hold = {}
            steps = []
            for dm in range(N_DM):
                def mid(dm=dm):
                    if dm == 0:
                        hold["ps"] = ps_small("ps_v")
                    nc.tensor.matmul(
                        hold["ps"][:],
                        xs[dm][:, sb * 128:(sb + 1) * 128],
                        w_sb["wv"][dm][:],
                        start=(dm == 0), stop=(dm == N_DM - 1),
                    )
                steps.append(mid)

            def out():  # bias add folded into the PSUM->SBUF copy
                nc.vector.tensor_tensor(
                    V[kb][:, :, 0:64],
                    hold["ps"][:].rearrange("p (h d) -> p h d", h=HG),
                    bvb[:].rearrange("p (h d) -> p h d", h=HG),
                    mybir.AluOpType.add,
                )
            steps.append(out)
            return steps

        def proj_chunk_steps(sc, xsmap, QTc):
            steps = []
            for p in range(N_PAIR):
                steps += proj_qk_steps("q", xsmap["q"], bq_sb, QTc[p], p)
            for p in range(N_PAIR):
                steps += proj_qk_steps("k", xsmap["k"], bk_sb, KT[p][sc], p)
            for sb in range(4):
                steps += proj_v_steps(xsmap["v"], sb, sc * 4 + sb)
            return steps

        def outproj_steps(qc, ctx_pairs):
            """Steps projecting chunk qc's normalized context to y[qc]."""
            steps = []
            for sb in range(4):
                hold = {}

                def mkyst(hold=hold):
                    hold["yst"] = ypool.tile([128, D], BF16, name="yst", tag="y")
                steps.append(mkyst)
                for dc in range(2):
                    for p in range(N_PAIR):
                        def mm(hold=hold, dc=dc, p=p, sb=sb):
                            if p == 0:
                                hold["yps"] = ps_small("yps")
                            nc.tensor.matmul(
                                hold["yps"][:],
                                ctx_pairs[p][:, sb * 128:(sb + 1) * 128],
                                wo_sb[p][:, dc * SC:(dc + 1) * SC],
                                start=(p == 0), stop=(p == N_PAIR - 1),
                            )
                        steps.append(mm)

                    def cp(hold=hold, dc=dc):
                        nc.vector.tensor_copy(
                            hold["yst"][:, dc * SC:(dc + 1) * SC], hold["yps"][:])
                    steps.append(cp)

                def store(hold=hold, sb=sb):
                    # interleave order puts all stage loads ahead of stores, so
                    # the Sync queue has no head-of-line risk
                    row = qc * SC + sb * 128
                    nc.sync.dma_start(y_out[row:row + 128, :], hold["yst"][:])
                steps.append(store)
            return steps

        # ---- attention for one q-chunk, with filler drip ----
        def attention(qc, QTc, filler, hold_back=0):
            kbmax = 4 * (qc + 1)
            blocks = [(p, kb) for p in range(N_PAIR) for kb in range(kbmax)]
            nb = len(blocks)
            drip_n = len(filler) - hold_back
            emitted = 0

            def drip(n):
                nonlocal emitted
                target = min(drip_n, (n + 1) * drip_n // nb)
                while emitted < target:
                    filler[emitted]()
                    emitted += 1

            scps_l = [None] * (nb + 1)
            ctx01 = {}
            ctx_pairs = [None] * N_PAIR

            def scores(n):
                p, kb = blocks[n]
                off = max(kb - 4 * qc, 0) * KB
                kt = KT[p][kb // 4]
                kcol = (kb % 4) * KB
                scps = pspool.tile([128, 2, SC], F32, name="scps", tag="pssc",
                                   bufs=2)
                nc.tensor.matmul(
                    scps[:, 0, off:SC], kt[0:64, kcol:kcol + KB],
                    QTc[p][0:64, off:SC], start=True, stop=True,
                )
                nc.tensor.matmul(
                    scps[:, 1, off:SC], kt[64:128, kcol:kcol + KB],
                    QTc[p][64:128, off:SC], start=True, stop=True,
                )
                scps_l[n] = scps

            scores(0)
            for n, (p, kb) in enumerate(blocks):
                j = kb - 4 * qc  # >=0: diagonal-crossing block
                off = max(j, 0) * KB  # live q-range is [off, SC)
                if kb == 0:
                    ctx01[p] = (
                        pspool.tile([128, SC], F32, name="ctx0", tag="psctx0",
                                    bufs=1),
                        pspool.tile([128, SC], F32, name="ctx1", tag="psctx1",
                                    bufs=1),
                    )
                scps = scps_l[n]
                e = epool.tile([128, 2, SC], BF16, name="e", tag="e", bufs=4)
                if off == 0:  # contiguous 2D view keeps ACT at full rate
                    nc.scalar.activation(
                        e[:].rearrange("p h s -> p (h s)"),
                        scps[:].rearrange("p h s -> p (h s)"),
                        mybir.ActivationFunctionType.Exp, scale=EXPSCALE,
                    )
                else:
                    nc.scalar.activation(
                        e[:, :, off:SC], scps[:, :, off:SC],
                        mybir.ActivationFunctionType.Exp, scale=EXPSCALE,
                    )
                if j >= 0:  # mask the leading 128-col triangle (both heads)
                    nc.vector.tensor_mul(e[:, :, off:off + KB],
                                         e[:, :, off:off + KB], mask_sb[:])
                # scores of the next block issue before this block's PV so the
                # PE isn't blocked on the exp, and ACT always has work queued
                if n + 1 < nb:
                    scores(n + 1)
                drip(n)
                first, last = kb == 0, kb == kbmax - 1
                ctx0, ctx1 = ctx01[p]
                nc.tensor.matmul(
                    ctx0[:, off:SC], V[kb][:, 2 * p, :], e[:, 0, off:SC],
                    start=first, stop=last,
                )
                nc.tensor.matmul(
                    ctx1[:, off:SC], V[kb][:, 2 * p + 1, :], e[:, 1, off:SC],
                    start=first, stop=last,
                )
                if last:
                    # normalize: ctx rows 0:64 / ctx row 64
                    cp = cpool.tile([128, SC], BF16, name="cp", tag="ctx")
                    final = qc == N_SC - 1 and p == N_PAIR - 1
                    if final:
                        # The last pair's norm gates the whole epilogue: the PE
                        # reorder window pulls the p3 outproj LDWEIGHTS ahead
                        # and head-of-line blocks on this chain. Normalize in
                        # 128-col blocks (den copies on the now-idle ACT) so
                        # outproj groups unblock column-by-column.
                        for cb in range(4):
                            cs = slice(cb * 128, (cb + 1) * 128)
                            for i, cps in ((0, ctx0), (1, ctx1)):
                                den = rpool.tile([64, 128], F32, name="den",
                                                 tag="recf", bufs=4)
                                nc.scalar.copy(den[:], cps[64:128, cs])
                                rec = rpool.tile([64, 128], F32, name="rec",
                                                 tag="recf", bufs=4)
                                nc.vector.reciprocal_approx_fast(rec[:], den[:])
                                nc.vector.tensor_tensor(
                                    cp[i * 64:(i + 1) * 64, cs], cps[0:64, cs],
                                    rec[:], mybir.AluOpType.mult,
                                )
                    else:
                        for i, cps in ((0, ctx0), (1, ctx1)):
                            den = rpool.tile([64, SC], F32, name="den", tag="rec",
                                             bufs=4)
                            # reciprocal mis-reads PSUM on HW: hop via SBUF
                            nc.vector.tensor_copy(den[:], cps[64:128, :])
                            rec = rpool.tile([64, SC], F32, name="rec", tag="rec",
                                             bufs=4)
                            nc.vector.reciprocal_approx_fast(rec[:], den[:])
                            nc.vector.tensor_tensor(
                                cp[i * 64:(i + 1) * 64, :], cps[0:64, :], rec[:],
                                mybir.AluOpType.mult,
                            )
                    ctx_pairs[p] = cp
            while emitted < len(filler):
                filler[emitted]()
                emitted += 1
            return ctx_pairs

        # ---- main pipeline ----
        stages = [stage_all(0), stage_all(1), None, None]
        QTcs = [[qtpool.tile([128, SC], BF16, name=f"QT{p}_{sc}", tag="qtc")
                 for p in range(N_PAIR)] for sc in range(N_SC)]
        # chunk-0 projections run standalone (nothing to overlap with yet)
        for st in proj_chunk_steps(0, stages[0], QTcs[0]):
            st()
        ctxs = [None] * N_SC
        for qc in range(N_SC):
            if qc + 2 < N_SC:
                stages[qc + 2] = stage_all(qc + 2)
            filler = []
            if qc == 1:
                filler += outproj_steps(0, ctxs[0])
            elif qc == 3:
                filler += outproj_steps(1, ctxs[1])
                filler += outproj_steps(2, ctxs[2])
            if qc + 1 < N_SC:
                filler += proj_chunk_steps(qc + 1, stages[qc + 1],
                                           QTcs[qc + 1])
            # hold back the tail of the last chunk's filler: it is emitted
            # after the final pair's norm, so the PE has work during the norm's
            # DVE chain and the HAM clock gate stays at 8/8 into the epilogue
            ctxs[qc] = attention(qc, QTcs[qc], filler,
                                 hold_back=24 if qc == N_SC - 1 else 0)
        for st in outproj_steps(N_SC - 1, ctxs[N_SC - 1]):
            st()

    nc.compile()
    return nc


def make_inputs(q, k, v, wq, bq, wk, bk, wv, bv, wo):
    """Host-side shard + layout prep. Returns list of 8 per-core input dicts."""
    qj = np.arange(KB)[None, :]
    ki = np.arange(KB)[:, None]
    mask = np.ascontiguousarray(
        np.repeat((qj >= ki).astype(NP_BF16)[:, None, :], 2, axis=1))

    def bt(a):  # bf16 contiguous
        return np.ascontiguousarray(np.asarray(a).astype(NP_BF16))

    qT = [bt(np.asarray(q[b]).T) for b in range(B)]
    kT = [bt(np.asarray(k[b]).T) for b in range(B)]
    vT = [bt(np.asarray(v[b]).T) for b in range(B)]

    in_maps = []
    for c in range(8):
        b, g = c // 2, c % 2
        sl = slice(g * DG, (g + 1) * DG)
        in_maps.append({
            "qT": qT[b], "kT": kT[b], "vT": vT[b],
            "wq": bt(wq[:, sl]),
            "wk": bt(wk[:, sl]),
            "wv": bt(wv[:, sl]),
            "wo": bt(wo[sl, :]),
            "bqT": np.ascontiguousarray(np.asarray(bq[sl], np.float32)).reshape(DG, 1),
            "bkT": np.ascontiguousarray(np.asarray(bk[sl], np.float32)).reshape(DG, 1),
            "bv": np.ascontiguousarray(np.asarray(bv[sl], np.float32)).reshape(1, DG),
            "masks": mask,
        })
    return in_maps


def combine_outputs(results, bo):
    """Sum the two row-parallel partials per batch and add the output bias."""
    out = np.empty((B, S, D), np.float32)
    for b in range(B):
        out[b] = (results[2 * b]["y"].astype(np.float32)
                  + results[2 * b + 1]["y"].astype(np.float32)
                  + np.asarray(bo, np.float32)[None, :])
    return out


_NC_CACHE = {}


def kernel(x, q, k, v, mask, wq, bq, wk, bk, wv, bv, wo, bo):
    # x is unused (overwritten in the reference forward); mask is the causal
    # tril mask, which is hardcoded in the on-device masking.
    if "nc" not in _NC_CACHE:
        _NC_CACHE["nc"] = build_program()
    nc = _NC_CACHE["nc"]
    in_maps = make_inputs(q, k, v, wq, bq, wk, bk, wv, bv, wo)
    out = None
    try:
        r = run_bass_kernel_spmd(nc, in_maps, core_ids=list(range(8)))
        out = combine_outputs(r.results, bo)
    except Exception:
        pass
    if out is None or not np.isfinite(out).all():
        # defensive: retry once on a transient exec failure or bad readback
        r = run_bass_kernel_spmd(nc, in_maps, core_ids=list(range(8)))
        out = combine_outputs(r.results, bo)
    return out
